# revision 35
# baseline (speedup 1.0000x reference)
"""GuidedAttentionLoss on 8 Trainium2 NeuronCores.

Math: loss = mean_b( sum_{f<F_b, l<L_b} A[b,f,l] * w[b,f,l] / F_b ),
      w = 1 - exp(-c*(l/L - f/F)^2),  c = 1/(2*gamma^(2*step)).

Key identity: exp(-c(x-y)^2) = exp(-cx^2)*exp(-cy^2)*exp(2cxy), and
exp(z) on z in [0, 2c) is approximated by a degree-D polynomial, so the
Gaussian weight is separable:  e[f,l] = sum_k h_k[f] * g_k[l]  with
  h_k[f] = a_k * (2c*y)^k * exp(-c*y^2),  y = f/F   (k = 0..D)
  g_k[l] = x^k * exp(-c*x^2),             x = l/L.
Then sum_{f,l} A*e = sum_k sum_l g_k[l] * C[k,l] with
  C[k,l] = sum_f h_k[f] * A[f,l]   -- a tiny-M matmul h^T @ A
(an extra all-ones column of h gives sum_f A for the "1" term).

The device kernel streams A through the TensorEngine against small
stationary weights, PSUM-accumulating a [16, L] result per batch; the
host does a tiny [16, L] f64 epilogue.

This version is tuned for the HBM roofline (the kernel is DMA-bound):
  * A is staged as fp8(e4m3): halves HBM traffic vs bf16. Element
    rounding (~3% rel) is unbiased and averages out over the ~1M-element
    contraction.
  * Weights h are fp8 with per-column power-of-2 scaling (undone in the
    host epilogue); the 3 dominant columns get an extra low-order fp8
    column (h = hi + lo), keeping the end-to-end loss error ~1e-5.
  * Matmuls run in DoubleRow perf mode: both operands fp8, each
    instruction contracts 256 rows (2 chunks) at 2x bf16 rate, so the
    PE (~10us) hides entirely under the DMA (~16us).
  * A is pre-transposed on the host to a partition-major flat stream,
    one contiguous HWDGE DMA per slot on the sync queue (h goes first on
    the same queue; the tiny outputs ride the scalar HWDGE queue). The
    last slot is fetched in small pieces so its matmuls chase the DMA.
  * Batch->slot assignment is optimized by hill-climbing swaps to
    minimize sum_slots max(chunks)*max(Lpad) (the transferred bytes);
    slots are processed largest-first so the post-DMA tail is short.
"""

import numpy as np
import ml_dtypes

import concourse.bass as bass  # noqa: F401
import concourse.tile as tile
from concourse import bacc, mybir
from concourse.bass_utils import run_bass_kernel_spmd

B, T_DEC, T_ENC = 64, 2048, 512
G_STEPS, GAMMA = 20000, 0.99995
N_CORES = 8
SLOTS = B // N_CORES
MC = 32          # stationary weight columns, padded (DoubleRow needs %16)
OC = 16          # output rows actually used (D+2+NLO <= OC <= MC)
NLO = 3          # number of leading columns that get a hi/lo split
FP8 = ml_dtypes.float8_e4m3   # TRN fp8e4: max normal 240
SCALE_TARGET = 200.0          # per-column max after power-of-2 scaling
TAIL_PAIRS = 1   # last slot DMA piece size (pairs) so MMs chase the DMA
N_WARM = 9       # dummy matmuls covering one ~3.4us HAM window, then real work


def _fit_exp_poly(zmax: float) -> np.ndarray:
    """Monomial coefficients a_k with exp(z) ~= sum a_k z^k on [0, zmax]."""
    from numpy.polynomial import chebyshev as C

    zs = np.linspace(0.0, zmax, 4001)
    ez = np.exp(zs)
    deg_max = MC - 2 - NLO
    for deg in range(8, deg_max + 1, 2):
        a = C.cheb2poly(C.chebfit(zs, ez, deg))
        err = np.max(np.abs(np.polynomial.polynomial.polyval(zs, a) - ez))
        if err < 1e-9 * np.exp(zmax):
            return a
    return a


def _plan(input_lengths: np.ndarray, target_lengths: np.ndarray):
    """Assign 64 batches to 8 slots x 8 cores, minimizing per-slot max work.

    Hill-climbs pairwise swaps from three sorted starts to minimize
    sum_slots max(chunks)*max(Lpad) == transferred A bytes. Slots are then
    ordered by descending cost (smallest slot last, so the post-DMA tail
    is short).
    """
    F = target_lengths.astype(np.int64)
    L = input_lengths.astype(np.int64)
    chunks = (F + 127) // 128
    Lp = np.minimum(T_ENC, -16 * (-L // 16))

    def cost(sb):
        return int((chunks[sb].max(1) * Lp[sb].max(1)).sum())

    rng = np.random.default_rng(0)
    best_c, best_sb = None, None
    orders = [
        np.argsort(-(chunks * Lp), kind="stable"),
        np.lexsort((-Lp, -chunks)),
        np.lexsort((-chunks, -Lp)),
    ]
    for order in orders:
        sb = np.stack([order[i * N_CORES:(i + 1) * N_CORES]
                       for i in range(SLOTS)])
        c0 = cost(sb)
        for _ in range(40000):
            i, j = rng.integers(0, SLOTS, 2)
            if i == j:
                continue
            a, b = rng.integers(0, N_CORES, 2)
            sb[i, a], sb[j, b] = sb[j, b], sb[i, a]
            c1 = cost(sb)
            if c1 <= c0:
                c0 = c1
            else:
                sb[i, a], sb[j, b] = sb[j, b], sb[i, a]
        if best_c is None or c0 < best_c:
            best_c, best_sb = c0, sb.copy()

    slots = [best_sb[i] for i in range(SLOTS)]
    costs = [int(chunks[s].max() * Lp[s].max()) for s in slots]
    order = np.argsort(-np.asarray(costs), kind="stable")
    sb = [slots[i] for i in order]
    sc = [int(chunks[s].max()) for s in sb]
    sl = [int(Lp[s].max()) for s in sb]
    return sb, sc, sl


def _slot_pieces(i, npr, trail):
    """DMA piece boundaries (in chunks) for slot i.

    The last slot is fetched in small pieces: its matmuls chase the DMA,
    and the completion-semaphore skew of the final piece (the last SDMA
    engine lags ~1us on a big transfer) shrinks with the piece."""
    nch = npr * 2 + trail
    if i < SLOTS - 1:
        return [(0, nch)]
    bounds = list(range(0, npr * 2, TAIL_PAIRS * 2)) + [nch]
    return list(zip(bounds, bounds[1:]))


def _build_program(slot_chunks, slot_L):
    f32 = mybir.dt.float32
    fp8 = mybir.dt.float8e4
    DR = mybir.MatmulPerfMode.DoubleRow

    npairs = [(nch + 1) // 2 for nch in slot_chunks]  # trailing odd chunk
    PTOT = sum(npairs)                                # occupies a pair slot
    TOT = sum(nch * Lm for nch, Lm in zip(slot_chunks, slot_L))

    nc = bacc.Bacc(
        "TRN2", target_bir_lowering=False, debug=False, num_devices=N_CORES
    )
    a_dr = nc.dram_tensor("a", [128, TOT], fp8, kind="ExternalInput")
    h_dr = nc.dram_tensor("h", [128, PTOT, 2, MC], fp8, kind="ExternalInput")
    bf16 = mybir.dt.bfloat16
    c_dr = [
        nc.dram_tensor(f"c{i}", [OC, slot_L[i]], bf16, kind="ExternalOutput")
        for i in range(SLOTS)
    ]

    with tile.TileContext(nc) as tc:
        with (
            tc.tile_pool(name="hp", bufs=1) as hpool,
            tc.tile_pool(name="ap", bufs=1) as apool,
            tc.tile_pool(name="op", bufs=1) as opool,
            tc.tile_pool(name="pp", bufs=4, space="PSUM") as pspool,
            tc.tile_pool(name="wp", bufs=1, space="PSUM") as wpool,
        ):
            # PE warmup: dummy DoubleRow matmuls on zeroed scratch keep the
            # HAM activity monitor busy from kernel start, so the real
            # matmuls run at 2.4 GHz instead of the 1.2 GHz cold clock.
            wsrc = hpool.tile([128, 1024], fp8, tag="wsrc")
            nc.gpsimd.memset(wsrc[:, :], 0)
            wps = wpool.tile([MC, 512], f32, tag="warm")
            wlhs = wsrc[:, 0:2 * MC].rearrange("p (two m) -> p two m", two=2)
            wrhs = wsrc[:, :].rearrange("p (two l) -> p two l", two=2)
            for _ in range(N_WARM):
                nc.tensor.matmul(wps[:, :], wlhs, wrhs, start=True, stop=True,
                                 perf_mode=DR)

            ht = hpool.tile([128, PTOT, 2, MC], fp8)
            nc.scalar.dma_start(ht[:, :, :, :], h_dr[:, :, :, :])
            ats = []   # per slot: list of (chunk0, piece tile)
            off = 0
            for i in range(SLOTS):
                nch, Lm = slot_chunks[i], slot_L[i]
                npr, trail = nch // 2, nch % 2
                pieces = []
                for kp, (c0, c1) in enumerate(_slot_pieces(i, npr, trail)):
                    at = apool.tile([128, (c1 - c0) * Lm], fp8,
                                    tag=f"a{i}_{kp}")
                    nc.sync.dma_start(
                        at[:, :], a_dr[:, off + c0 * Lm:off + c1 * Lm]
                    )
                    pieces.append((c0, at))
                ats.append(pieces)
                off += nch * Lm
            poff = 0
            for i in range(SLOTS):
                nch, Lm = slot_chunks[i], slot_L[i]
                npr, trail = nch // 2, nch % 2
                ps = pspool.tile([MC, Lm], f32, tag="ps")

                def piece_at(ch):
                    for c0, t in reversed(ats[i]):
                        if ch >= c0:
                            return c0, t
                    raise AssertionError

                for j in range(npr):
                    c0, at = piece_at(2 * j)
                    o = (2 * j - c0) * Lm
                    rhs = at[:, o:o + 2 * Lm].rearrange(
                        "p (two l) -> p two l", two=2
                    )
                    nc.tensor.matmul(
                        ps[:, :],
                        ht[:, poff + j, :, :],
                        rhs,
                        start=(j == 0),
                        stop=(j == npr - 1 and not trail),
                        perf_mode=DR,
                    )
                if trail:
                    c0, at = piece_at(nch - 1)
                    o = (nch - 1 - c0) * Lm
                    nc.tensor.matmul(
                        ps[:, :],
                        ht[:, poff + npr, 0, :],
                        at[:, o:o + Lm],
                        start=(npr == 0),
                        stop=True,
                    )
                poff += npr + trail
                ot = opool.tile([OC, Lm], bf16, tag=f"o{i}")
                if i == SLOTS - 1:
                    # pipeline the tail: copy+DMA in two halves
                    half = (Lm // 2 + 15) // 16 * 16
                    for s0, s1 in ((0, half), (half, Lm)):
                        nc.vector.tensor_copy(ot[:, s0:s1], ps[:OC, s0:s1])
                        nc.scalar.dma_start(c_dr[i][:, s0:s1], ot[:, s0:s1])
                else:
                    nc.vector.tensor_copy(ot[:, :], ps[:OC, :])
                    nc.scalar.dma_start(c_dr[i][:, :], ot[:, :])
    nc.compile()
    return nc


def _kernel_impl(alignments, input_lengths, target_lengths, global_step,
                 trace=False):
    step = int(global_step)
    if G_STEPS < step:
        return np.zeros((), dtype=np.float32), None

    g = GAMMA ** step
    c = 1.0 / (2.0 * g * g)
    a_poly = _fit_exp_poly(2.0 * c)
    D = len(a_poly) - 1
    assert D + 2 + NLO <= OC

    F = target_lengths.astype(np.int64)
    L = input_lengths.astype(np.int64)
    slot_batches, slot_chunks, slot_L = _plan(input_lengths, target_lengths)
    npairs = [(nch + 1) // 2 for nch in slot_chunks]
    poffs = np.concatenate([[0], np.cumsum(npairs)]).astype(int)
    PTOT = int(poffs[-1])
    TOT = sum(nch * Lm for nch, Lm in zip(slot_chunks, slot_L))

    nc = _build_program(slot_chunks, slot_L)

    al = np.asarray(alignments, dtype=np.float32)
    in_maps = []
    scales = []  # [core][slot] -> (hi_s[D+2], lo_s[NLO])
    for j in range(N_CORES):
        a_all = np.empty((128, TOT), dtype=FP8)
        h_all = np.zeros((128, PTOT, 2, MC), dtype=FP8)
        ssc = []
        off = 0
        for i in range(SLOTS):
            b = int(slot_batches[i][j])
            nch, Lm = slot_chunks[i], slot_L[i]
            R = nch * 128
            blk = al[b, :R, :Lm].astype(FP8)
            a_all[:, off:off + nch * Lm] = (
                blk.reshape(nch, 128, Lm).transpose(1, 0, 2).reshape(128, -1)
            )
            off += nch * Lm

            Fb = int(F[b])
            y = np.arange(R, dtype=np.float64) / Fb
            h = np.zeros((R, MC), dtype=np.float64)
            for k in range(D + 1):
                h[:, k] = a_poly[k] * (2.0 * c * y) ** k * np.exp(-c * y * y)
            h[:, D + 1] = 1.0
            h[Fb:, :] = 0.0
            hi_s = np.ones(D + 2)
            lo_s = np.ones(NLO)
            hsc = np.zeros((R, MC), dtype=np.float64)
            for k in range(D + 2):
                m = np.abs(h[:, k]).max()
                if m > 0:
                    hi_s[k] = 2.0 ** np.floor(np.log2(SCALE_TARGET / m))
                hsc[:, k] = h[:, k] * hi_s[k]
            hi8 = hsc[:, :D + 2].astype(FP8)
            for k in range(NLO):
                r = hsc[:, k] - hi8[:, k].astype(np.float64)
                mr = np.abs(r).max()
                if mr > 0:
                    lo_s[k] = 2.0 ** np.floor(np.log2(SCALE_TARGET / mr))
                hsc[:, D + 2 + k] = r * lo_s[k]
            h8 = hsc.astype(FP8)
            h8[:, :D + 2] = hi8
            # pack rows into pair-chunks: ht[p, poff+jj, t, :] = h8[jj*256+t*128+p]
            hp = np.zeros((npairs[i] * 256, MC), dtype=FP8)
            hp[:R] = h8
            h_all[:, poffs[i]:poffs[i + 1], :, :] = (
                hp.reshape(npairs[i], 2, 128, MC).transpose(2, 0, 1, 3)
            )
            ssc.append((hi_s, lo_s))
        in_maps.append({"a": a_all, "h": h_all})
        scales.append(ssc)

    res = run_bass_kernel_spmd(nc, in_maps, list(range(N_CORES)), trace=trace)

    # Host epilogue: tiny [OC, L] combinations per batch, f64.
    per_sample = np.zeros(B, dtype=np.float64)
    for j in range(N_CORES):
        for i in range(SLOTS):
            b = int(slot_batches[i][j])
            Lb = int(L[b])
            hi_s, lo_s = scales[j][i]
            Cm = res.results[j][f"c{i}"].astype(np.float64)
            Ck = Cm[:D + 1, :Lb] / hi_s[:D + 1, None]
            Ck[:NLO] += (Cm[D + 2:D + 2 + NLO, :Lb]
                         / (hi_s[:NLO, None] * lo_s[:, None]))
            sA = Cm[D + 1, :Lb] / hi_s[D + 1]
            x = np.arange(Lb, dtype=np.float64) / Lb
            ex = np.exp(-c * x * x)
            gsum = np.zeros(Lb)
            xk = np.ones(Lb)
            for k in range(D + 1):
                gsum += Ck[k] * xk
                xk *= x
            per_sample[b] = sA.sum() - (gsum * ex).sum()
    loss = np.float64(np.mean(per_sample / F.astype(np.float64)))
    return np.asarray(loss, dtype=np.float32), res


def kernel(alignments, input_lengths, target_lengths, global_step):
    loss, _ = _kernel_impl(alignments, input_lengths, target_lengths,
                           global_step)
    return loss


# revision 37
# speedup vs baseline: 1.0704x; 1.0704x over previous
"""GuidedAttentionLoss on 8 Trainium2 NeuronCores.

Math: loss = mean_b( sum_{f<F_b, l<L_b} A[b,f,l] * w[b,f,l] / F_b ),
      w = 1 - exp(-c*(l/L - f/F)^2),  c = 1/(2*gamma^(2*step)).

Key identity: exp(-c(x-y)^2) = exp(-cx^2)*exp(-cy^2)*exp(2cxy), and
exp(z) on z in [0, 2c) is approximated by a degree-D polynomial, so the
Gaussian weight is separable:  e[f,l] = sum_k h_k[f] * g_k[l]  with
  h_k[f] = a_k * (2c*y)^k * exp(-c*y^2),  y = f/F   (k = 0..D)
  g_k[l] = x^k * exp(-c*x^2),             x = l/L.
Then sum_{f,l} A*e = sum_k sum_l g_k[l] * C[k,l] with
  C[k,l] = sum_f h_k[f] * A[f,l]   -- a tiny-M matmul h^T @ A
(an extra all-ones column of h gives sum_f A for the "1" term).

The device kernel streams A through the TensorEngine against small
stationary weights, PSUM-accumulating a [16, L] result per batch; the
host does a tiny [16, L] f64 epilogue.

This version is tuned for the HBM roofline (the kernel is DMA-bound):
  * A is staged as fp8(e4m3): halves HBM traffic vs bf16. Element
    rounding (~3% rel) is unbiased and averages out over the ~1M-element
    contraction.
  * Weights h are fp8 with per-column power-of-2 scaling (undone in the
    host epilogue); the 3 dominant columns get an extra low-order fp8
    column (h = hi + lo), keeping the end-to-end loss error ~1e-5.
  * Matmuls run in DoubleRow perf mode: both operands fp8, each
    instruction contracts 256 rows (2 chunks) at 2x bf16 rate, so the
    PE (~10us) hides entirely under the DMA (~16us).
  * A is pre-transposed on the host to a partition-major flat stream,
    one contiguous HWDGE DMA per slot on the sync queue (h goes first on
    the same queue; the tiny outputs ride the scalar HWDGE queue). The
    last slot is fetched in small pieces so its matmuls chase the DMA.
  * Batch->slot assignment is optimized by hill-climbing swaps to
    minimize sum_slots max(chunks)*max(Lpad) (the transferred bytes);
    slots are processed largest-first so the post-DMA tail is short.
"""

import numpy as np
import ml_dtypes

import concourse.bass as bass  # noqa: F401
import concourse.tile as tile
from concourse import bacc, mybir
from concourse.bass_utils import run_bass_kernel_spmd

B, T_DEC, T_ENC = 64, 2048, 512
G_STEPS, GAMMA = 20000, 0.99995
N_CORES = 8
SLOTS = B // N_CORES
MC = 32          # stationary weight columns, padded (DoubleRow needs %16)
OC = 16          # output rows actually used (D+2+NLO <= OC <= MC)
NLO = 3          # number of leading columns that get a hi/lo split
FP8 = ml_dtypes.float8_e4m3   # TRN fp8e4: max normal 240
SCALE_TARGET = 200.0          # per-column max after power-of-2 scaling
TAIL_PAIRS = 1   # last slot DMA piece size (pairs) so MMs chase the DMA
N_WARM = 9       # dummy matmuls covering one ~3.4us HAM window, then real work


def _fit_exp_poly(zmax: float) -> np.ndarray:
    """Monomial coefficients a_k with exp(z) ~= sum a_k z^k on [0, zmax]."""
    from numpy.polynomial import chebyshev as C

    zs = np.linspace(0.0, zmax, 4001)
    ez = np.exp(zs)
    deg_max = MC - 2 - NLO
    for deg in range(8, deg_max + 1, 2):
        a = C.cheb2poly(C.chebfit(zs, ez, deg))
        err = np.max(np.abs(np.polynomial.polynomial.polyval(zs, a) - ez))
        if err < 1e-9 * np.exp(zmax):
            return a
    return a


def _plan(input_lengths: np.ndarray, target_lengths: np.ndarray):
    """Assign 64 batches to 8 slots x 8 cores, minimizing per-slot max work.

    Hill-climbs pairwise swaps from three sorted starts to minimize
    sum_slots max(chunks)*max(Lpad) == transferred A bytes. Slots are then
    ordered by descending cost (smallest slot last, so the post-DMA tail
    is short).
    """
    F = target_lengths.astype(np.int64)
    L = input_lengths.astype(np.int64)
    chunks = (F + 127) // 128
    Lp = np.minimum(T_ENC, -16 * (-L // 16))

    def cost(sb):
        return int((chunks[sb].max(1) * Lp[sb].max(1)).sum())

    rng = np.random.default_rng(0)
    best_c, best_sb = None, None
    orders = [
        np.argsort(-(chunks * Lp), kind="stable"),
        np.lexsort((-Lp, -chunks)),
        np.lexsort((-chunks, -Lp)),
    ]
    for order in orders:
        sb = np.stack([order[i * N_CORES:(i + 1) * N_CORES]
                       for i in range(SLOTS)])
        c0 = cost(sb)
        for _ in range(40000):
            i, j = rng.integers(0, SLOTS, 2)
            if i == j:
                continue
            a, b = rng.integers(0, N_CORES, 2)
            sb[i, a], sb[j, b] = sb[j, b], sb[i, a]
            c1 = cost(sb)
            if c1 <= c0:
                c0 = c1
            else:
                sb[i, a], sb[j, b] = sb[j, b], sb[i, a]
        if best_c is None or c0 < best_c:
            best_c, best_sb = c0, sb.copy()

    slots = [best_sb[i] for i in range(SLOTS)]
    costs = [int(chunks[s].max() * Lp[s].max()) for s in slots]
    order = np.argsort(-np.asarray(costs), kind="stable")
    sb = [slots[i] for i in order]
    sc = [int(chunks[s].max()) for s in sb]
    sl = [int(Lp[s].max()) for s in sb]
    return sb, sc, sl


def _slot_pieces(i, npr, trail):
    """DMA piece boundaries (in chunks) for slot i.

    The last slot is fetched in small pieces: its matmuls chase the DMA,
    and the completion-semaphore skew of the final piece (the last SDMA
    engine lags ~1us on a big transfer) shrinks with the piece."""
    nch = npr * 2 + trail
    if i < SLOTS - 1:
        return [(0, nch)]
    bounds = list(range(0, npr * 2, TAIL_PAIRS * 2)) + [nch]
    return list(zip(bounds, bounds[1:]))


def _build_program(slot_chunks, slot_L):
    f32 = mybir.dt.float32
    fp8 = mybir.dt.float8e4
    DR = mybir.MatmulPerfMode.DoubleRow

    npairs = [(nch + 1) // 2 for nch in slot_chunks]  # trailing odd chunk
    PTOT = sum(npairs)                                # occupies a pair slot
    TOT = sum(nch * Lm for nch, Lm in zip(slot_chunks, slot_L))

    nc = bacc.Bacc(
        "TRN2", target_bir_lowering=False, debug=False, num_devices=N_CORES
    )
    a_dr = nc.dram_tensor("a", [128, TOT], fp8, kind="ExternalInput")
    h_dr = nc.dram_tensor("h", [128, PTOT, 2, MC], fp8, kind="ExternalInput")
    c_dr = [
        nc.dram_tensor(f"c{i}", [OC, slot_L[i]], f32, kind="ExternalOutput")
        for i in range(SLOTS)
    ]

    with tile.TileContext(nc) as tc:
        with (
            tc.tile_pool(name="hp", bufs=1) as hpool,
            tc.tile_pool(name="ap", bufs=1) as apool,
            tc.tile_pool(name="op", bufs=1) as opool,
            tc.tile_pool(name="pp", bufs=4, space="PSUM") as pspool,
            tc.tile_pool(name="wp", bufs=1, space="PSUM") as wpool,
        ):
            # PE warmup: dummy DoubleRow matmuls on zeroed scratch keep the
            # HAM activity monitor busy from kernel start, so the real
            # matmuls run at 2.4 GHz instead of the 1.2 GHz cold clock.
            wsrc = hpool.tile([128, 1024], fp8, tag="wsrc")
            nc.gpsimd.memset(wsrc[:, :], 0)
            wps = wpool.tile([MC, 512], f32, tag="warm")
            wlhs = wsrc[:, 0:2 * MC].rearrange("p (two m) -> p two m", two=2)
            wrhs = wsrc[:, :].rearrange("p (two l) -> p two l", two=2)
            for _ in range(N_WARM):
                nc.tensor.matmul(wps[:, :], wlhs, wrhs, start=True, stop=True,
                                 perf_mode=DR)

            ht = hpool.tile([128, PTOT, 2, MC], fp8)
            nc.scalar.dma_start(ht[:, :, :, :], h_dr[:, :, :, :])
            ats = []   # per slot: list of (chunk0, piece tile)
            off = 0
            for i in range(SLOTS):
                nch, Lm = slot_chunks[i], slot_L[i]
                npr, trail = nch // 2, nch % 2
                pieces = []
                for kp, (c0, c1) in enumerate(_slot_pieces(i, npr, trail)):
                    at = apool.tile([128, (c1 - c0) * Lm], fp8,
                                    tag=f"a{i}_{kp}")
                    nc.sync.dma_start(
                        at[:, :], a_dr[:, off + c0 * Lm:off + c1 * Lm]
                    )
                    pieces.append((c0, at))
                ats.append(pieces)
                off += nch * Lm
            poff = 0
            for i in range(SLOTS):
                nch, Lm = slot_chunks[i], slot_L[i]
                npr, trail = nch // 2, nch % 2
                ps = pspool.tile([MC, Lm], f32, tag="ps")

                def piece_at(ch):
                    for c0, t in reversed(ats[i]):
                        if ch >= c0:
                            return c0, t
                    raise AssertionError

                for j in range(npr):
                    c0, at = piece_at(2 * j)
                    o = (2 * j - c0) * Lm
                    rhs = at[:, o:o + 2 * Lm].rearrange(
                        "p (two l) -> p two l", two=2
                    )
                    nc.tensor.matmul(
                        ps[:, :],
                        ht[:, poff + j, :, :],
                        rhs,
                        start=(j == 0),
                        stop=(j == npr - 1 and not trail),
                        perf_mode=DR,
                    )
                if trail:
                    c0, at = piece_at(nch - 1)
                    o = (nch - 1 - c0) * Lm
                    nc.tensor.matmul(
                        ps[:, :],
                        ht[:, poff + npr, 0, :],
                        at[:, o:o + Lm],
                        start=(npr == 0),
                        stop=True,
                    )
                poff += npr + trail
                ot = opool.tile([OC, Lm], f32, tag=f"o{i}")
                nc.vector.tensor_copy(ot[:, :], ps[:OC, :])
                nc.scalar.dma_start(c_dr[i][:, :], ot[:, :])
    nc.compile()
    return nc


def _kernel_impl(alignments, input_lengths, target_lengths, global_step,
                 trace=False):
    step = int(global_step)
    if G_STEPS < step:
        return np.zeros((), dtype=np.float32), None

    g = GAMMA ** step
    c = 1.0 / (2.0 * g * g)
    a_poly = _fit_exp_poly(2.0 * c)
    D = len(a_poly) - 1
    assert D + 2 + NLO <= OC

    F = target_lengths.astype(np.int64)
    L = input_lengths.astype(np.int64)
    slot_batches, slot_chunks, slot_L = _plan(input_lengths, target_lengths)
    npairs = [(nch + 1) // 2 for nch in slot_chunks]
    poffs = np.concatenate([[0], np.cumsum(npairs)]).astype(int)
    PTOT = int(poffs[-1])
    TOT = sum(nch * Lm for nch, Lm in zip(slot_chunks, slot_L))

    nc = _build_program(slot_chunks, slot_L)

    al = np.asarray(alignments, dtype=np.float32)
    in_maps = []
    scales = []  # [core][slot] -> (hi_s[D+2], lo_s[NLO])
    for j in range(N_CORES):
        a_all = np.empty((128, TOT), dtype=FP8)
        h_all = np.zeros((128, PTOT, 2, MC), dtype=FP8)
        ssc = []
        off = 0
        for i in range(SLOTS):
            b = int(slot_batches[i][j])
            nch, Lm = slot_chunks[i], slot_L[i]
            R = nch * 128
            blk = al[b, :R, :Lm].astype(FP8)
            a_all[:, off:off + nch * Lm] = (
                blk.reshape(nch, 128, Lm).transpose(1, 0, 2).reshape(128, -1)
            )
            off += nch * Lm

            Fb = int(F[b])
            y = np.arange(R, dtype=np.float64) / Fb
            h = np.zeros((R, MC), dtype=np.float64)
            for k in range(D + 1):
                h[:, k] = a_poly[k] * (2.0 * c * y) ** k * np.exp(-c * y * y)
            h[:, D + 1] = 1.0
            h[Fb:, :] = 0.0
            hi_s = np.ones(D + 2)
            lo_s = np.ones(NLO)
            hsc = np.zeros((R, MC), dtype=np.float64)
            for k in range(D + 2):
                m = np.abs(h[:, k]).max()
                if m > 0:
                    hi_s[k] = 2.0 ** np.floor(np.log2(SCALE_TARGET / m))
                hsc[:, k] = h[:, k] * hi_s[k]
            hi8 = hsc[:, :D + 2].astype(FP8)
            for k in range(NLO):
                r = hsc[:, k] - hi8[:, k].astype(np.float64)
                mr = np.abs(r).max()
                if mr > 0:
                    lo_s[k] = 2.0 ** np.floor(np.log2(SCALE_TARGET / mr))
                hsc[:, D + 2 + k] = r * lo_s[k]
            h8 = hsc.astype(FP8)
            h8[:, :D + 2] = hi8
            # pack rows into pair-chunks: ht[p, poff+jj, t, :] = h8[jj*256+t*128+p]
            hp = np.zeros((npairs[i] * 256, MC), dtype=FP8)
            hp[:R] = h8
            h_all[:, poffs[i]:poffs[i + 1], :, :] = (
                hp.reshape(npairs[i], 2, 128, MC).transpose(2, 0, 1, 3)
            )
            ssc.append((hi_s, lo_s))
        in_maps.append({"a": a_all, "h": h_all})
        scales.append(ssc)

    res = run_bass_kernel_spmd(nc, in_maps, list(range(N_CORES)), trace=trace)

    # Host epilogue: tiny [OC, L] combinations per batch, f64.
    per_sample = np.zeros(B, dtype=np.float64)
    for j in range(N_CORES):
        for i in range(SLOTS):
            b = int(slot_batches[i][j])
            Lb = int(L[b])
            hi_s, lo_s = scales[j][i]
            Cm = res.results[j][f"c{i}"].astype(np.float64)
            Ck = Cm[:D + 1, :Lb] / hi_s[:D + 1, None]
            Ck[:NLO] += (Cm[D + 2:D + 2 + NLO, :Lb]
                         / (hi_s[:NLO, None] * lo_s[:, None]))
            sA = Cm[D + 1, :Lb] / hi_s[D + 1]
            x = np.arange(Lb, dtype=np.float64) / Lb
            ex = np.exp(-c * x * x)
            gsum = np.zeros(Lb)
            xk = np.ones(Lb)
            for k in range(D + 1):
                gsum += Ck[k] * xk
                xk *= x
            per_sample[b] = sA.sum() - (gsum * ex).sum()
    loss = np.float64(np.mean(per_sample / F.astype(np.float64)))
    return np.asarray(loss, dtype=np.float32), res


def kernel(alignments, input_lengths, target_lengths, global_step):
    loss, _ = _kernel_impl(alignments, input_lengths, target_lengths,
                           global_step)
    return loss
